# revision 18
# baseline (speedup 1.0000x reference)
"""Trainium2 Bass kernel for a 4-layer dense transformer (GQA + RoPE +
gated MLP + tied lm_head), distributed over 8 NeuronCores.

Sharding: sequence-parallel over tokens for the transformer body (core r
owns tokens t with t % 8 == r), with one AllGather of (roped K, V) per
layer; the tied lm_head is vocab-parallel (each core computes all 2048
tokens x 4000 vocab columns) after one AllGather of the final normed
activations.

All matmuls run as float32r (FP22 multiply, FP32 accumulate) which is
full PE speed for moving dims >= 256. Activations are kept TRANSPOSED
([feature, token]) so no per-layer transposes are needed; RMSNorm weight
vectors are folded into the adjacent weight matrices on the host.
"""

import os
import numpy as np

# ---- model dims (hardcoded per problem spec) ----
V, D, H, KV, L, FF, HD = 32000, 768, 12, 4, 4, 2048, 64
T = 2048
NCORES = 8
TO = T // NCORES          # 256 tokens owned per core
VS = V // NCORES          # 4000 vocab columns per core
EPS = 1e-6
ROPE_THETA = 10000.0
SCALE = 1.0 / np.sqrt(HD)
DK = D // 128             # 6 k-tiles over D
FK = FF // 128            # 16 tiles over FF
KT = T // 128             # 16 key tiles over gathered T
# head pairing: q-head pairs whose kv-heads land on partition halves (0,64)
# of the gathered kT tiles.  kv(h) = h // 3.
PERM = [0, 3, 1, 4, 2, 5, 6, 9, 7, 10, 8, 11]

_CACHE = {}


def _build_nc(n_layers):
    import concourse.bass as bass
    import concourse.bacc as bacc
    import concourse.tile as tile
    from concourse import mybir
    from concourse.masks import make_identity

    F32 = mybir.dt.float32
    F32R = mybir.dt.float32r
    I32 = mybir.dt.int32
    AF = mybir.ActivationFunctionType

    nc = bacc.Bacc(None, target_bir_lowering=False, num_devices=NCORES)

    # ---------------- I/O ----------------
    emb_g = nc.dram_tensor("emb_g", [V, D], F32, kind="ExternalInput")
    idx_own = nc.dram_tensor("idx_own", [2, 128, 1], I32, kind="ExternalInput")
    # transposed-layout rope tables, duplicated over a head pair (128 rows);
    # sin2 carries the rotate-half sign (-sin on rows d%64<32, +sin above)
    cos_t = nc.dram_tensor("cos_t", [128, TO], F32, kind="ExternalInput")
    sin_t = nc.dram_tensor("sin_t", [128, TO], F32, kind="ExternalInput")
    mask_in = nc.dram_tensor("mask_in", [KT, 128, TO], F32, kind="ExternalInput")
    wq_in = nc.dram_tensor("wq_in", [L, DK, 128, H * HD], F32, kind="ExternalInput")
    wk_in = nc.dram_tensor("wk_in", [L, DK, 128, KV * HD], F32, kind="ExternalInput")
    wv_in = nc.dram_tensor("wv_in", [L, DK, 128, KV * HD], F32, kind="ExternalInput")
    wo_in = nc.dram_tensor("wo_in", [L, DK, 128, D], F32, kind="ExternalInput")
    wg_in = nc.dram_tensor("wg_in", [L, FK, 128, D], F32, kind="ExternalInput")
    wu_in = nc.dram_tensor("wu_in", [L, FK, 128, D], F32, kind="ExternalInput")
    wd_in = nc.dram_tensor("wd_in", [L, FK, 128, D], F32, kind="ExternalInput")
    embst = nc.dram_tensor("embst", [DK, 128, VS], F32, kind="ExternalInput")
    logits_sh = nc.dram_tensor("logits_sh", [T, VS], F32, kind="ExternalOutput")

    RG = [list(range(NCORES))]

    with tile.TileContext(nc) as tc:
        with (
            tc.tile_pool(name="persist", bufs=1) as persist,
            tc.tile_pool(name="dram", bufs=2, space="DRAM") as dram,
        ):
            # persistent tiles
            ident = persist.tile([128, 128], F32)
            make_identity(nc, ident[:])
            ones_f32 = persist.tile([128, KV], F32)
            nc.vector.memset(ones_f32[:], 1.0)
            ones_col = persist.tile([128, 1], F32R)
            nc.vector.tensor_copy(ones_col[:], ones_f32[:, 0:1])
            eps_sc = persist.tile([1, 1], F32)
            nc.vector.memset(eps_sc[:], EPS)
            xT = [persist.tile([128, TO], F32, name=f"xT{k}") for k in range(DK)]

            with tc.tile_pool(name="lay", bufs=1) as lay:
                # constants for the layer phase
                cos_sb = lay.tile([128, TO], F32)
                nc.sync.dma_start(cos_sb[:], cos_t[:])
                sin_sb = lay.tile([128, TO], F32)
                nc.sync.dma_start(sin_sb[:], sin_t[:])
                mask_sb = []
                for kt in range(KT):
                    m = lay.tile([128, TO], F32, name=f"mask{kt}", tag="mask",
                                 bufs=KT)
                    nc.sync.dma_start(m[:], mask_in[kt])
                    mask_sb.append(m)

                with tc.tile_pool(name="ps", bufs=1, space="PSUM") as ps:
                    # ---- embedding gather + transpose into xT ----
                    for tt in range(2):
                        idx_sb = lay.tile([128, 1], I32, tag="idx", bufs=2)
                        nc.sync.dma_start(idx_sb[:], idx_own[tt])
                        x0 = lay.tile([128, D], F32, tag="x0", bufs=2)
                        nc.gpsimd.indirect_dma_start(
                            out=x0[:],
                            out_offset=None,
                            in_=emb_g[:],
                            in_offset=bass.IndirectOffsetOnAxis(
                                ap=idx_sb[:, :1], axis=0),
                        )
                        for k in range(DK):
                            tr = ps.tile([128, 128], F32, tag="acc", bufs=6)
                            nc.tensor.transpose(
                                tr[:], x0[:, k * 128:(k + 1) * 128], ident[:])
                            nc.vector.tensor_copy(
                                xT[k][:, tt * 128:(tt + 1) * 128], tr[:])

                    def rmsnorm(tag):
                        """Compute x_hat = x * rstd(x) (ln weight folded into
                        the next matmul's weights on the host)."""
                        ssq = ps.tile([1, TO], F32, tag="sm", bufs=2)
                        for k in range(DK):
                            sq = lay.tile([128, TO], F32R, tag="sq", bufs=2)
                            nc.scalar.square(sq[:], xT[k][:])
                            nc.tensor.matmul(
                                ssq[:], ones_col[:], sq[:],
                                start=(k == 0), stop=(k == DK - 1))
                        s2 = lay.tile([1, TO], F32, tag="s2", bufs=2)
                        nc.scalar.activation(
                            s2[:], ssq[:], AF.Sqrt, bias=eps_sc[:], scale=1.0 / D)
                        rstd = lay.tile([1, TO], F32, tag="rstd", bufs=2)
                        nc.vector.reciprocal(rstd[:], s2[:])
                        rstd_b = lay.tile([128, TO], F32, tag="rstd_b", bufs=2)
                        nc.gpsimd.partition_broadcast(rstd_b[:], rstd[0:1, :])
                        xh = []
                        for k in range(DK):
                            t = lay.tile([128, TO], F32R,
                                         name=f"xh_{tag}_{k}", tag="xh", bufs=DK)
                            nc.vector.tensor_mul(t[:], xT[k][:], rstd_b[:])
                            xh.append(t)
                        return xh

                    def rope(dst, src_ps):
                        """dst[128, TO] = rope(src_ps[128, TO]) for a stacked
                        pair of 64-dim heads.  rotate_half's cross-partition
                        move is done with 4 SBUF->SBUF DMA shifts; the sign
                        lives in the sin table."""
                        qf = lay.tile([128, TO], F32, tag="ropef", bufs=2)
                        nc.vector.tensor_copy(qf[:], src_ps[:])
                        qsw = lay.tile([128, TO], F32, tag="ropes", bufs=2)
                        for h0 in (0, 64):
                            nc.sync.dma_start(
                                qsw[h0:h0 + 32, :], qf[h0 + 32:h0 + 64, :])
                            nc.sync.dma_start(
                                qsw[h0 + 32:h0 + 64, :], qf[h0:h0 + 32, :])
                        tmp = lay.tile([128, TO], F32, tag="rtmp", bufs=2)
                        nc.vector.tensor_mul(tmp[:], qsw[:], sin_sb[:])
                        nc.vector.tensor_mul(dst[:], qf[:], cos_sb[:])
                        nc.vector.tensor_add(dst[:], dst[:], tmp[:])

                    for l in range(n_layers):
                        # ================= attention =================
                        xh = rmsnorm(f"a{l}")

                        # K: project + rope, ship to collective input
                        cc_kv_in = dram.tile([2, 256, 256], F32, tag="kvin",
                                             bufs=2)
                        wk_sb = []
                        for k in range(DK):
                            w = lay.tile([128, KV * HD], F32R,
                                         name=f"wk{k}", tag="wk", bufs=DK)
                            nc.sync.dma_start(w[:], wk_in[l, k].bitcast(F32R))
                            wk_sb.append(w)
                        for g in range(2):
                            k_ps = ps.tile([128, TO], F32, tag="acc", bufs=6)
                            for k in range(DK):
                                nc.tensor.matmul(
                                    k_ps[:], wk_sb[k][:, g * 128:(g + 1) * 128],
                                    xh[k][:],
                                    start=(k == 0), stop=(k == DK - 1))
                            kT_own = lay.tile([128, TO], F32, tag="kTo", bufs=2)
                            rope(kT_own[:], k_ps[:])
                            nc.sync.dma_start(
                                cc_kv_in[0, g * 128:(g + 1) * 128, :], kT_own[:])

                        # V: project in [token, kv_dim] layout, ship
                        wv_sb = []
                        for k in range(DK):
                            w = lay.tile([128, KV * HD], F32R,
                                         name=f"wv{k}", tag="wv", bufs=DK)
                            nc.sync.dma_start(w[:], wv_in[l, k].bitcast(F32R))
                            wv_sb.append(w)
                        for tt in range(2):
                            v_ps = ps.tile([128, KV * HD], F32, tag="acc", bufs=6)
                            for k in range(DK):
                                nc.tensor.matmul(
                                    v_ps[:],
                                    xh[k][:, tt * 128:(tt + 1) * 128],
                                    wv_sb[k][:],
                                    start=(k == 0), stop=(k == DK - 1))
                            v_sb = lay.tile([128, KV * HD], F32, tag="vsb",
                                            bufs=2)
                            nc.vector.tensor_copy(v_sb[:], v_ps[:])
                            nc.sync.dma_start(
                                cc_kv_in[1, tt * 128:(tt + 1) * 128, :], v_sb[:])

                        cc_kv_out = dram.tile([NCORES, 2, 256, 256], F32,
                                              tag="kvout", bufs=2,
                                              addr_space="Shared")
                        nc.gpsimd.collective_compute(
                            "AllGather", mybir.AluOpType.bypass,
                            replica_groups=RG,
                            ins=[cc_kv_in[:]], outs=[cc_kv_out[:]])

                        # Q: project + rope (overlaps the collective)
                        wq_sb = []
                        for k in range(DK):
                            w = lay.tile([128, H * HD], F32R,
                                         name=f"wq{k}", tag="wq", bufs=DK)
                            nc.sync.dma_start(w[:], wq_in[l, k].bitcast(F32R))
                            wq_sb.append(w)
                        qT = []
                        for hp in range(6):
                            q_ps = ps.tile([128, TO], F32, tag="acc", bufs=6)
                            for k in range(DK):
                                nc.tensor.matmul(
                                    q_ps[:],
                                    wq_sb[k][:, hp * 128:(hp + 1) * 128],
                                    xh[k][:],
                                    start=(k == 0), stop=(k == DK - 1))
                            qt = lay.tile([128, TO], F32R,
                                          name=f"qT{hp}", tag="qT", bufs=6)
                            rope(qt[:], q_ps[:])
                            qT.append(qt)

                        # gathered K^T: [128(2 kv heads), T] x 2
                        kTf = []
                        for dkv in range(2):
                            kt_t = lay.tile([128, T], F32R,
                                            name=f"kTf{dkv}", tag="kTf", bufs=2)
                            src = cc_kv_out[:, 0, dkv * 128:(dkv + 1) * 128, :]
                            nc.sync.dma_start(
                                kt_t[:].rearrange("r (s c) -> r s c", s=NCORES),
                                src.rearrange("s r c -> r s c").bitcast(F32R))
                            kTf.append(kt_t)
                        # gathered V as [128 keys, kv_head, HD+1] per key tile
                        v5 = []
                        for kt in range(KT):
                            vt = lay.tile([128, KV, 1 + HD], F32R,
                                          name=f"v5_{kt}", tag="v5", bufs=KT)
                            s, half = kt // 2, kt % 2
                            nc.sync.dma_start(
                                vt[:, :, 1:1 + HD],
                                cc_kv_out[s, 1, half * 128:(half + 1) * 128, :]
                                .rearrange("r (g d) -> r g d", g=KV)
                                .bitcast(F32R))
                            nc.vector.tensor_copy(vt[:, :, 0:1], ones_f32[:])
                            v5.append(vt)

                        # attention per head pair
                        attnT = []
                        for hp in range(6):
                            at = lay.tile([128, TO], F32R,
                                          name=f"attnT{hp}", tag="attnT", bufs=6)
                            attnT.append(at)
                        for hp in range(6):
                            dkv = hp // 3
                            for rt in range(2):
                                head = PERM[2 * hp + rt]
                                kvh = head // 3
                                off = rt * 64
                                exps = []
                                for kt in range(KT):
                                    s_ps = ps.tile([128, TO], F32, tag="acc",
                                                   bufs=6)
                                    nc.tensor.matmul(
                                        s_ps[:],
                                        kTf[dkv][off:off + 64,
                                                 kt * 128:(kt + 1) * 128],
                                        qT[hp][off:off + 64, :],
                                        start=True, stop=True,
                                        tile_position=(off, 0))
                                    e = lay.tile([128, TO], F32R,
                                                 tag="exps", bufs=8)
                                    nc.scalar.activation(
                                        e[:], s_ps[:], AF.Exp, scale=SCALE)
                                    nc.vector.tensor_mul(
                                        e[:], e[:], mask_sb[kt][:].bitcast(F32R))
                                    exps.append(e)
                                a_ps = ps.tile([1 + HD, TO], F32, tag="sm",
                                               bufs=2)
                                for kt in range(KT):
                                    nc.tensor.matmul(
                                        a_ps[:], v5[kt][:, kvh, :], exps[kt][:],
                                        start=(kt == 0), stop=(kt == KT - 1))
                                rec = lay.tile([1, TO], F32, tag="rec", bufs=4)
                                nc.vector.reciprocal(rec[:], a_ps[0:1, :])
                                rec_b = lay.tile([1 + HD, TO], F32, tag="rec_b",
                                                 bufs=4)
                                nc.gpsimd.partition_broadcast(
                                    rec_b[:], rec[0:1, :])
                                anrm = lay.tile([1 + HD, TO], F32R, tag="anrm",
                                                bufs=4)
                                nc.vector.tensor_mul(
                                    anrm[:], a_ps[:], rec_b[:])
                                nc.sync.dma_start(
                                    attnT[hp][off:off + 64, :],
                                    anrm[1:1 + HD, :])

                        # output projection + residual
                        wo_sb = []
                        for k in range(DK):
                            w = lay.tile([128, D], F32R,
                                         name=f"wo{k}", tag="wo", bufs=DK)
                            nc.sync.dma_start(w[:], wo_in[l, k].bitcast(F32R))
                            wo_sb.append(w)
                        for dk in range(DK):
                            xo_ps = ps.tile([128, TO], F32, tag="acc", bufs=6)
                            for k in range(DK):
                                nc.tensor.matmul(
                                    xo_ps[:],
                                    wo_sb[k][:, dk * 128:(dk + 1) * 128],
                                    attnT[k][:],
                                    start=(k == 0), stop=(k == DK - 1))
                            nc.vector.tensor_add(xT[dk][:], xo_ps[:], xT[dk][:])

                        # ================= gated MLP =================
                        xh = rmsnorm(f"m{l}")
                        guT = []
                        for ff in range(FK):
                            wg_sb = lay.tile([128, D], F32R, tag="wg", bufs=2)
                            nc.sync.dma_start(wg_sb[:], wg_in[l, ff].bitcast(F32R))
                            wu_sb = lay.tile([128, D], F32R, tag="wu", bufs=2)
                            nc.sync.dma_start(wu_sb[:], wu_in[l, ff].bitcast(F32R))
                            g_ps = ps.tile([128, TO], F32, tag="acc", bufs=6)
                            for k in range(DK):
                                nc.tensor.matmul(
                                    g_ps[:], wg_sb[:, k * 128:(k + 1) * 128],
                                    xh[k][:],
                                    start=(k == 0), stop=(k == DK - 1))
                            g_act = lay.tile([128, TO], F32, tag="gact", bufs=3)
                            nc.scalar.activation(
                                g_act[:], g_ps[:], AF.Gelu_apprx_tanh)
                            u_ps = ps.tile([128, TO], F32, tag="acc", bufs=6)
                            for k in range(DK):
                                nc.tensor.matmul(
                                    u_ps[:], wu_sb[:, k * 128:(k + 1) * 128],
                                    xh[k][:],
                                    start=(k == 0), stop=(k == DK - 1))
                            gu = lay.tile([128, TO], F32R,
                                          name=f"guT{ff}", tag="guT", bufs=FK)
                            nc.vector.tensor_mul(gu[:], g_act[:], u_ps[:])
                            guT.append(gu)

                        xd_ps = [ps.tile([128, TO], F32, name=f"xd{dk}",
                                         tag="acc", bufs=6)
                                 for dk in range(DK)]
                        for ff in range(FK):
                            wd_sb = lay.tile([128, D], F32R, tag="wd", bufs=2)
                            nc.sync.dma_start(wd_sb[:], wd_in[l, ff].bitcast(F32R))
                            for dk in range(DK):
                                nc.tensor.matmul(
                                    xd_ps[dk][:],
                                    wd_sb[:, dk * 128:(dk + 1) * 128],
                                    guT[ff][:],
                                    start=(ff == 0), stop=(ff == FK - 1))
                        for dk in range(DK):
                            nc.vector.tensor_add(
                                xT[dk][:], xd_ps[dk][:], xT[dk][:])

                    # ---- final norm + ship own activations ----
                    xh = rmsnorm("f")
                    cc_xf_in = dram.tile([D, TO], F32, bufs=1)
                    for k in range(DK):
                        nc.sync.dma_start(
                            cc_xf_in[k * 128:(k + 1) * 128, :].bitcast(F32R),
                            xh[k][:])
                    cc_xf_out = dram.tile([NCORES, D, TO], F32, bufs=1,
                                          addr_space="Shared")
                    nc.gpsimd.collective_compute(
                        "AllGather", mybir.AluOpType.bypass,
                        replica_groups=RG,
                        ins=[cc_xf_in[:]], outs=[cc_xf_out[:]])

            # ================= lm_head (vocab shard) =================
            with (
                tc.tile_pool(name="lm", bufs=1) as lm,
                tc.tile_pool(name="pslm", bufs=1, space="PSUM") as pslm,
            ):
                xfT = []
                for k in range(DK):
                    t = lm.tile([128, T], F32R, name=f"xfT{k}", tag="xfT",
                                bufs=DK)
                    nc.sync.dma_start(
                        t[:].rearrange("r (s c) -> r s c", s=NCORES),
                        cc_xf_out[:, k * 128:(k + 1) * 128, :]
                        .rearrange("s r c -> r s c").bitcast(F32R))
                    xfT.append(t)

                vchunks = []
                off = 0
                while off < VS:
                    w = min(512, VS - off)
                    vchunks.append((off, w))
                    off += w
                for (voff, vw) in vchunks:
                    emb_sb = []
                    for k in range(DK):
                        e = lm.tile([128, 512], F32R, tag="embsb", bufs=2 * DK)
                        nc.sync.dma_start(
                            e[:, 0:vw], embst[k, :, voff:voff + vw].bitcast(F32R))
                        emb_sb.append(e)
                    for tt in range(T // 128):
                        o_ps = pslm.tile([128, 512], F32, tag="lmh", bufs=8)
                        for k in range(DK):
                            nc.tensor.matmul(
                                o_ps[:, 0:vw],
                                xfT[k][:, tt * 128:(tt + 1) * 128],
                                emb_sb[k][:, 0:vw],
                                start=(k == 0), stop=(k == DK - 1))
                        o_sb = lm.tile([128, 512], F32, tag="osb", bufs=8)
                        nc.vector.tensor_copy(o_sb[:, 0:vw], o_ps[:, 0:vw])
                        nc.sync.dma_start(
                            logits_sh[tt * 128:(tt + 1) * 128, voff:voff + vw],
                            o_sb[:, 0:vw])

    nc.finalize()
    return nc


def _get_nc(n_layers=L):
    key = ("nc", n_layers)
    if key not in _CACHE:
        _CACHE[key] = _build_nc(n_layers)
    return _CACHE[key]


def _host_prep(idx, embed, ln1_w, Wq, Wk, Wv, Wo, ln2_w, Wg, Wu, Wd, normf_w):
    """Fold norms into weights, permute heads, pre-tile for DMA, build
    per-core input maps."""
    f32 = np.float32
    idx = np.asarray(idx).reshape(T).astype(np.int32)
    embed = np.asarray(embed, f32)
    Wq = np.asarray(Wq, f32) * np.asarray(ln1_w, f32)[:, :, None]
    Wk = np.asarray(Wk, f32) * np.asarray(ln1_w, f32)[:, :, None]
    Wv = np.asarray(Wv, f32) * np.asarray(ln1_w, f32)[:, :, None]
    Wg_ = np.asarray(Wg, f32) * np.asarray(ln2_w, f32)[:, :, None]
    Wu_ = np.asarray(Wu, f32) * np.asarray(ln2_w, f32)[:, :, None]
    Wo = np.asarray(Wo, f32)
    Wd = np.asarray(Wd, f32)

    # permute q heads and Wo rows so paired heads' kv-halves line up
    Wq = Wq.reshape(L, D, H, HD)[:, :, PERM, :].reshape(L, D, H * HD)
    Wo = Wo.reshape(L, H, HD, D)[:, PERM, :, :].reshape(L, H * HD, D)

    wq_t = np.ascontiguousarray(Wq.reshape(L, DK, 128, H * HD))
    wk_t = np.ascontiguousarray(Wk.reshape(L, DK, 128, KV * HD))
    wv_t = np.ascontiguousarray(Wv.reshape(L, DK, 128, KV * HD))
    wo_t = np.ascontiguousarray(Wo.reshape(L, DK, 128, D))
    # [L, D, FF] -> [L, FK, 128, D] with element [l, f, r, b*128+c] =
    # W[l, b*128+r, f*128+c]
    wg_t = np.ascontiguousarray(
        Wg_.reshape(L, DK, 128, FK, 128).transpose(0, 3, 2, 1, 4)
        .reshape(L, FK, 128, D))
    wu_t = np.ascontiguousarray(
        Wu_.reshape(L, DK, 128, FK, 128).transpose(0, 3, 2, 1, 4)
        .reshape(L, FK, 128, D))
    wd_t = np.ascontiguousarray(Wd.reshape(L, FK, 128, D))

    emb_scaled_T = np.ascontiguousarray(
        (embed * np.asarray(normf_w, f32)[None, :]).T)   # [D, V]

    inv_freq = (1.0 / (ROPE_THETA **
                       (np.arange(0, HD, 2, dtype=np.float64) / HD)))  # [32]

    in_maps = []
    for r in range(NCORES):
        own_pos = np.arange(r, T, NCORES, dtype=np.float64)  # [TO]
        freqs = own_pos[None, :] * inv_freq[:, None]          # [32, TO]
        c32 = np.cos(freqs)
        s32 = np.sin(freqs)
        # [128, TO]: rows = head-pair dims; sin sign folds rotate_half
        cos_r = np.tile(c32, (4, 1)).astype(f32)
        sin_r = np.tile(np.concatenate([-s32, s32], axis=0), (2, 1)).astype(f32)
        # mask: gathered key (s, jj) visible to own query i iff
        # 8*jj + s <= 8*i + r
        s = np.arange(T) // TO
        jj = np.arange(T) % TO
        pos_k = 8 * jj + s                      # [T] gathered key positions
        pos_q = 8 * np.arange(TO) + r           # [TO]
        mask = (pos_k[:, None] <= pos_q[None, :]).astype(f32)  # [T, TO]
        mask_t = np.ascontiguousarray(mask.reshape(KT, 128, TO))
        in_maps.append({
            "emb_g": embed,
            "idx_own": np.ascontiguousarray(
                idx[r::NCORES].reshape(2, 128, 1)),
            "cos_t": cos_r,
            "sin_t": sin_r,
            "mask_in": mask_t,
            "wq_in": wq_t, "wk_in": wk_t, "wv_in": wv_t, "wo_in": wo_t,
            "wg_in": wg_t, "wu_in": wu_t, "wd_in": wd_t,
            "embst": np.ascontiguousarray(
                emb_scaled_T[:, r * VS:(r + 1) * VS].reshape(DK, 128, VS)),
        })
    return in_maps


def _assemble(results):
    full = np.concatenate([r["logits_sh"] for r in results], axis=1)  # [T,V]
    # gathered row g = s*256 + jj corresponds to global token 8*jj + s
    logits = full.reshape(NCORES, TO, V).transpose(1, 0, 2).reshape(T, V)
    return np.ascontiguousarray(logits)[None]  # [1, T, V]


def kernel(idx, embed, ln1_w, Wq, Wk, Wv, Wo, ln2_w, Wg, Wu, Wd, normf_w):
    from concourse.bass_utils import run_bass_kernel_spmd

    in_maps = _host_prep(idx, embed, ln1_w, Wq, Wk, Wv, Wo, ln2_w,
                         Wg, Wu, Wd, normf_w)
    nc = _get_nc(int(os.environ.get("BODY_LAYERS", L)))
    res = run_bass_kernel_spmd(nc, in_maps, core_ids=list(range(NCORES)))
    return _assemble(res.results)
